# revision 4
# baseline (speedup 1.0000x reference)
"""Trainium2 Bass kernel for 16-head MHA (B=2, S=2048, D=1024, E=64).

Sharding: 8 cores = 2 batches x 4 head-groups. Each core computes 4 heads
(2 pairs of 2) for one batch and returns a partial output [2048, 1024]
(sum of its 4 heads' contributions after the output projection). Host sums
the 4 partials per batch.

Per-core pipeline (all matmuls on PE, fp32 PSUM accumulation):
  - projections QT/KT (feature-major, fp16 in, fp32r out), V (fp16 in,
    transposed on PE to token-major fp32r [V|1] tiles)
  - S^T = K Q^T per head pair, two heads row-packed in the 128x128 array
  - A^T = exp(S^T) on ACT (scale folded into W_query on host), fp32r
  - O^T accumulation with fused row-sum via the [V|1] ones column
  - softmax normalization: DVE reciprocal + GPSIMD partition-broadcast +
    DVE multiply (writes fp16 O^T)
  - output projection (fp16) accumulating both pairs, DMA out token-major
"""

import sys

sys.path.insert(0, "/opt/trn_rl_repo")

import numpy as np

import concourse.bass as bass
import concourse.bacc as bacc
import concourse.mybir as mybir
from concourse import tile
from concourse.bass_interp import get_hw_module
from concourse.bass_utils import run_bass_kernel_spmd

F16 = mybir.dt.float16
F32 = mybir.dt.float32
F32R = mybir.dt.float32r

N_CORES = 8
T = 2048          # tokens per core (one batch)
D = 1024          # model dim
E = 64            # head dim
QC = 512          # query chunk
NQ = T // QC      # 4 query chunks
KB = 128          # key block
NKB = T // KB     # 16 key blocks
ND = D // 128     # 8 contraction chunks for projections

_CACHE = {}


def _build():
    nc = bacc.Bacc("TRN2", target_bir_lowering=False, debug=False,
                   num_devices=N_CORES)

    xqT = nc.dram_tensor("xqT", [D, T], F16, kind="ExternalInput").ap()
    xkT = nc.dram_tensor("xkT", [D, T], F16, kind="ExternalInput").ap()
    xvT = nc.dram_tensor("xvT", [D, T], F16, kind="ExternalInput").ap()
    # per-pair packed weights, layout [128, 8*128]: chunk d at cols d*128
    wq = [nc.dram_tensor(f"wq{p}", [128, D], F16, kind="ExternalInput").ap()
          for p in range(2)]
    wk = [nc.dram_tensor(f"wk{p}", [128, D], F16, kind="ExternalInput").ap()
          for p in range(2)]
    wv = [nc.dram_tensor(f"wv{p}", [128, D], F16, kind="ExternalInput").ap()
          for p in range(2)]
    wo = [nc.dram_tensor(f"wo{p}", [128, D], F16, kind="ExternalInput").ap()
          for p in range(2)]
    ident_d = nc.dram_tensor("ident", [128, 128], F32, kind="ExternalInput").ap()
    pout = nc.dram_tensor("pout", [T, D], F32, kind="ExternalOutput").ap()

    with tile.TileContext(nc) as tc:
        with (
            tc.tile_pool(name="consts", bufs=1) as consts,
            tc.tile_pool(name="persist", bufs=1) as persist,
            tc.tile_pool(name="xs", bufs=8) as xs,
            tc.tile_pool(name="at", bufs=3) as atp,
            tc.tile_pool(name="o2t", bufs=2) as o2tp,
            tc.tile_pool(name="os", bufs=3) as osp,
            tc.tile_pool(name="small", bufs=4) as smallp,
        ):
            # ---- constants ----
            wq_sb = [consts.tile([128, D], F16, tag=f"wq{p}", name=f"wq_sb{p}") for p in range(2)]
            wk_sb = [consts.tile([128, D], F16, tag=f"wk{p}", name=f"wk_sb{p}") for p in range(2)]
            wv_sb = [consts.tile([128, D], F16, tag=f"wv{p}", name=f"wv_sb{p}") for p in range(2)]
            wo_sb = [consts.tile([128, D], F16, tag=f"wo{p}", name=f"wo_sb{p}") for p in range(2)]
            ident = consts.tile([128, 128], F32, tag="ident")
            for p in range(2):
                nc.sync.dma_start(wq_sb[p][:], wq[p][:])
                nc.sync.dma_start(wk_sb[p][:], wk[p][:])
                nc.sync.dma_start(wv_sb[p][:], wv[p][:])
                nc.sync.dma_start(wo_sb[p][:], wo[p][:])
            nc.sync.dma_start(ident[:], ident_d[:])

            # ---- persistent activations ----
            # feature-major Q^T, K^T per pair: rows 0:64 head0, 64:128 head1
            qt = [persist.tile([128, T], F32R, tag=f"qt{p}", name=f"qt{p}") for p in range(2)]
            kt = [persist.tile([128, T], F32R, tag=f"kt{p}", name=f"kt{p}") for p in range(2)]
            # token-major [V | 1] per head: 16 blocks of [128, 65]
            v2 = [persist.tile([128, NKB * 65], F32R, tag=f"v2{h}", name=f"v2_{h}")
                  for h in range(4)]
            ones_f32 = consts.tile([128, NKB], F32, tag="ones", name="ones_f32")
            nc.vector.memset(ones_f32[:], 1.0)
            for h in range(4):
                ones_ap = v2[h][:].rearrange("p (b c) -> p b c", c=65)[:, :, 64:65]
                nc.vector.tensor_copy(ones_ap, ones_f32[:].rearrange("p (b o) -> p b o", o=1))

            # ---- phase 1: projections ----
            with (
                tc.tile_pool(name="psA", bufs=3, space="PSUM") as psA,
                tc.tile_pool(name="psT", bufs=1, space="PSUM") as psT,
                tc.tile_pool(name="vtmp", bufs=1) as vtmp,
            ):
                def project(x_dram, w_sb, out_tiles, evac):
                    # out_tiles[p][:, t*QC:+QC] accumulated over ND chunks
                    for t in range(NQ):
                        xt = [None] * ND
                        for d in range(ND):
                            xt[d] = xs.tile([128, QC], F16, tag="x", name=f"x_{t}_{d}")
                            nc.sync.dma_start(
                                xt[d][:], x_dram[d * 128:(d + 1) * 128,
                                                 t * QC:(t + 1) * QC])
                        for p in range(2):
                            ps = psA.tile([128, QC], F32, tag="proj", name=f"proj_{t}_{p}")
                            for d in range(ND):
                                nc.tensor.matmul(
                                    ps[:], w_sb[p][:, d * 128:(d + 1) * 128],
                                    xt[d][:], start=(d == 0), stop=(d == ND - 1))
                            evac(out_tiles, p, t, ps)

                def evac_act(out_tiles, p, t, ps):
                    nc.scalar.activation(out_tiles[p][:, t * QC:(t + 1) * QC],
                                         ps[:], mybir.ActivationFunctionType.Copy)

                # K first, then V (+ transpose), then Q — lets attention start
                # as soon as possible while Q tiles still stream.
                project(xkT, wk_sb, kt, evac_act)

                vt = [vtmp.tile([128, T], F32, tag=f"vt{p}", name=f"vt{p}") for p in range(2)]

                def evac_dve(out_tiles, p, t, ps):
                    nc.vector.tensor_copy(out_tiles[p][:, t * QC:(t + 1) * QC],
                                          ps[:])

                project(xvT, wv_sb, vt, evac_dve)

                # transpose V to token-major [V|1] tiles
                for p in range(2):
                    for blk in range(NKB):
                        pt = psT.tile([128, 128], F32, tag="tr", name=f"tr_{p}_{blk}")
                        nc.tensor.transpose(
                            pt[:], vt[p][:, blk * 128:(blk + 1) * 128], ident[:])
                        nc.vector.tensor_copy(
                            v2[2 * p][:, blk * 65:blk * 65 + 64], pt[:, 0:64])
                        nc.vector.tensor_copy(
                            v2[2 * p + 1][:, blk * 65:blk * 65 + 64],
                            pt[:, 64:128])

                project(xqT, wq_sb, qt, evac_act)

            # ---- phase 2: attention + output projection ----
            with (
                tc.tile_pool(name="psS", bufs=2, space="PSUM") as psS,
                tc.tile_pool(name="psO", bufs=1, space="PSUM") as psO,
                tc.tile_pool(name="psP", bufs=2, space="PSUM") as psP,
            ):
                for qc in range(NQ):
                    q0 = qc * QC
                    o2t = [o2tp.tile([128, QC], F16, tag=f"o2t{p}", name=f"o2t_{qc}_{p}")
                           for p in range(2)]
                    for p in range(2):
                        po = [psO.tile([65, QC], F32, tag=f"o{h}", name=f"po_{qc}_{p}_{h}")
                              for h in range(2)]
                        for kb in range(NKB):
                            k0 = kb * KB
                            ps = psS.tile([128, 2 * QC], F32, tag="s", name=f"s_{qc}_{p}_{kb}")
                            nc.tensor.matmul(
                                ps[:, 0:QC],
                                kt[p][0:64, k0:k0 + KB],
                                qt[p][0:64, q0:q0 + QC],
                                start=True, stop=True, tile_position=(0, 0))
                            nc.tensor.matmul(
                                ps[:, QC:2 * QC],
                                kt[p][64:128, k0:k0 + KB],
                                qt[p][64:128, q0:q0 + QC],
                                start=True, stop=True, tile_position=(64, 0))
                            at = atp.tile([128, 2 * QC], F32R, tag="at", name=f"at_{qc}_{p}_{kb}")
                            nc.scalar.activation(
                                at[:], ps[:], mybir.ActivationFunctionType.Exp)
                            for h in range(2):
                                nc.tensor.matmul(
                                    po[h][:],
                                    v2[2 * p + h][:, kb * 65:kb * 65 + 65],
                                    at[:, h * QC:(h + 1) * QC],
                                    start=(kb == 0), stop=(kb == NKB - 1))
                        for h in range(2):
                            r = smallp.tile([1, QC], F32, tag="r", name=f"r_{qc}_{p}_{h}")
                            nc.vector.reciprocal(r[:], po[h][64:65, :])
                            rb = smallp.tile([64, QC], F32, tag="rb", name=f"rb_{qc}_{p}_{h}")
                            nc.gpsimd.partition_broadcast(rb[:], r[:])
                            nc.vector.tensor_mul(
                                o2t[p][h * 64:(h + 1) * 64, :],
                                po[h][0:64, :], rb[:])
                    # output projection for this query chunk
                    for sub in range(4):
                        ost = osp.tile([128, D], F32, tag="os", name=f"os_{qc}_{sub}")
                        for oc in range(2):
                            pp = psP.tile([128, 512], F32, tag="pp", name=f"pp_{qc}_{sub}_{oc}")
                            for p in range(2):
                                nc.tensor.matmul(
                                    pp[:],
                                    o2t[p][:, sub * 128:(sub + 1) * 128],
                                    wo_sb[p][:, oc * 512:(oc + 1) * 512],
                                    start=(p == 0), stop=(p == 1))
                            nc.vector.tensor_copy(
                                ost[:, oc * 512:(oc + 1) * 512], pp[:])
                        nc.sync.dma_start(
                            pout[q0 + sub * 128:q0 + (sub + 1) * 128, :],
                            ost[:])

    nc.compile()
    nc.m = get_hw_module(nc.m)
    return nc


def _pack_w(w_pair):
    # w_pair: [2, 1024, 64] -> [1024, 128] -> chunk-major [128, 8*128]
    w = np.concatenate([w_pair[0], w_pair[1]], axis=1)          # [1024, 128]
    return np.ascontiguousarray(
        w.reshape(ND, 128, 128).transpose(1, 0, 2).reshape(128, D))


def _pack_wo(wo_pair):
    # wo_pair: [2, 64, 1024] -> [128, 1024]
    return np.ascontiguousarray(np.concatenate([wo_pair[0], wo_pair[1]], axis=0))


def kernel(q, k, v, W_query, W_key, W_val, W_out, _trace=False):
    q = np.asarray(q, dtype=np.float32)
    k = np.asarray(k, dtype=np.float32)
    v = np.asarray(v, dtype=np.float32)
    W_query = np.asarray(W_query, dtype=np.float32)
    W_key = np.asarray(W_key, dtype=np.float32)
    W_val = np.asarray(W_val, dtype=np.float32)
    W_out = np.asarray(W_out, dtype=np.float32)

    if "nc" not in _CACHE:
        _CACHE["nc"] = _build()
    nc = _CACHE["nc"]

    norm = 1.0 / np.sqrt(E)
    ident = np.eye(128, dtype=np.float32)
    xT = {}
    for b in range(2):
        xT[("q", b)] = np.ascontiguousarray(q[b].T).astype(np.float16)
        xT[("k", b)] = np.ascontiguousarray(k[b].T).astype(np.float16)
        xT[("v", b)] = np.ascontiguousarray(v[b].T).astype(np.float16)

    in_maps = []
    for c in range(N_CORES):
        b, g = c // 4, c % 4
        hs = [4 * g, 4 * g + 1, 4 * g + 2, 4 * g + 3]
        m = {
            "xqT": xT[("q", b)], "xkT": xT[("k", b)], "xvT": xT[("v", b)],
            "ident": ident,
        }
        for p in range(2):
            hp = hs[2 * p:2 * p + 2]
            m[f"wq{p}"] = _pack_w(W_query[hp] * norm).astype(np.float16)
            m[f"wk{p}"] = _pack_w(W_key[hp]).astype(np.float16)
            m[f"wv{p}"] = _pack_w(W_val[hp]).astype(np.float16)
            m[f"wo{p}"] = _pack_wo(W_out[hp]).astype(np.float16)
        in_maps.append(m)

    res = run_bass_kernel_spmd(nc, in_maps, list(range(N_CORES)),
                               trace=_trace)
    parts = [res.results[c]["pout"] for c in range(N_CORES)]
    out = np.stack([
        parts[0] + parts[1] + parts[2] + parts[3],
        parts[4] + parts[5] + parts[6] + parts[7],
    ]).astype(np.float32)
    if _trace:
        _CACHE["last_result"] = res
    return out
